# revision 32
# baseline (speedup 1.0000x reference)
"""CategoryDense (nn_CategoryDense) TRN2 Bass kernel — bf16 with
host-side transpose layout and phase-separated DMA.

out[b, c, o] = sum_i x[b, c, i] * kernel[0, c, i, o] + bias[0, c, o]
x: [8192, 64, 64] f32; kernel: [1, 64, 64, 64]; bias: [1, 64, 64].

Data-parallel over 8 NeuronCores: batch dim sharded 1024 rows/core,
weights + bias replicated; no cross-core communication.

The problem is HBM-bound. All device I/O is bf16 (host casts x down and
the result back up; rel-err budget 2e-2, ~5.6e-3 measured), halving DMA
bytes vs f32 — and bf16 matmuls run 1 cycle/row on PE where f32r needs
4 at 128-wide output.  The host upload stores x pre-transposed per
128-row b-tile as xt[t, p, j, b] = x[128t+b, 128j+p] (p = contraction
index of category pair j), so matmul lhsT tiles stream straight from
HBM at full descriptor size — no PE transposes, no PSUM round trip.

Device structure per core (8 b-tiles x 4 [128,1024] 2-bank PSUM
groups; all x and out tiles SBUF-resident):
  - PE runs ONLY the 256 [128,128] bf16 matmuls vs the block-diagonal
    weight stacks (the DMA feed paces PE at its mid p-state, so every
    extra PE cycle costs double — bias never touches PE).
  - Drains: ~2/3 of groups ACT-copy PSUM->SBUF with DVE adding bias
    afterwards in cheap all-SBUF 2x mode; the rest are fused DVE
    PSUM adds.  Bias arrives HOST-pre-broadcast as a plain [128, CO]
    load (an on-chip partition_broadcast is a 7.5us serial GPSIMD op
    that late-gates every drain).
  - DMA is strictly phase-separated: all 8 x tiles load upfront on the
    SP HWDGE ring at full bandwidth; stores ride SWDGE behind a tiny
    Pool gate that reads the second-to-last x tile, so the store
    stream provably cannot contend with the compute-pacing loads.
    Tiles 6-7 store via the (by then idle) SP ring so the last bytes
    don't queue at the back of the SWDGE FIFO.
"""

from contextlib import ExitStack

import numpy as np
import ml_dtypes

import concourse.bass as bass  # noqa: F401  (engine namespaces live on nc)
import concourse.mybir as mybir
import concourse.tile as tile
from concourse import bacc
from concourse.bass_utils import run_bass_kernel_spmd

F32 = mybir.dt.float32
BF16 = mybir.dt.bfloat16
NP_BF16 = ml_dtypes.bfloat16

N_CORES = 8
B, C, IN, OUT = 8192, 64, 64, 64
B_SHARD = B // N_CORES
N_PAIRS = C // 2          # 32 category pairs; one [128,128] matmul each
CI = C * IN               # 4096
CO = C * OUT              # 4096
N_BTILES = B_SHARD // 128  # 8
GROUPS = 4                # [128,1024] PSUM groups per b-tile (8 pairs each)
SKEW = 2                  # matmul emission lag behind psum alloc/prefill
# Units g with g % 8 in ACT_PHASE are ACT-drained (bias prefilled by
# ACT), the rest DVE-drained (bias added inline).  12/32 on ACT.
ACT_PHASE = (1, 4, 6)


def _build_nc(b_shard=B_SHARD):
    n_btiles = b_shard // 128
    total = n_btiles * GROUPS
    nc = bacc.Bacc("TRN2", target_bir_lowering=False, debug=False)
    # Host-pre-transposed x: xt[t, p, j, b] = x[128t+b, 128j+p].
    xt = nc.dram_tensor("xt", [n_btiles, 128, N_PAIRS, 128], BF16,
                        kind="ExternalInput").ap()
    # Host-prepared block-diagonal weight stacks (see kernel() below).
    wall = nc.dram_tensor("wall", [128, N_PAIRS, 128], BF16,
                          kind="ExternalInput").ap()
    # Host-pre-broadcast bias: [128, CO], identical rows.  A plain 1MB
    # HWDGE load; the on-chip partition_broadcast alternative is a
    # 7.5us serial GPSIMD op that late-gates every drain behind it.
    biasf = nc.dram_tensor("biasf", [128, CO], BF16,
                           kind="ExternalInput").ap()
    out = nc.dram_tensor("out", [b_shard, C, OUT], BF16,
                         kind="ExternalOutput").ap()

    out_t = out.rearrange("(t p) c o -> t p (c o)", p=128)

    with tile.TileContext(nc) as tc, ExitStack() as ctx:
        const_pool = ctx.enter_context(tc.tile_pool(name="const", bufs=1))
        x_pool = ctx.enter_context(tc.tile_pool(name="x", bufs=8))
        out_pool = ctx.enter_context(tc.tile_pool(name="out", bufs=8))
        # Four 2-bank PSUM tiles = all 8 banks.
        psum_o = ctx.enter_context(
            tc.tile_pool(name="psum_o", bufs=4, space="PSUM"))

        # Constants on the ACT HWDGE ring.  Few, large DMAs: each
        # dma_start costs ~650ns of sequencer + sem-lane time, and the 8
        # HWDGE completion lanes are shared with the load ring, so a
        # chain of small const DMAs head-of-line blocks the x loads.
        w_all = const_pool.tile([128, N_PAIRS, 128], BF16)
        for k in range(2):
            nc.scalar.dma_start(w_all[:, 16 * k:16 * (k + 1)],
                                wall[:, 16 * k:16 * (k + 1)])
        # Bias last: the first matmuls need w (and x0), while tile 0's
        # drains are bias-free ACT copies — bias isn't read until ~25us.
        bias_sb = const_pool.tile([128, CO], BF16)
        nc.scalar.dma_start(bias_sb[:], biasf[:])

        def load_tile(t):
            x_sb = x_pool.tile([128, N_PAIRS, 128], BF16, tag="x_sb",
                               name=f"x_sb_{t}")
            if t == 0:
                for h in range(2):
                    nc.sync.dma_start(x_sb[:, h * 16:(h + 1) * 16],
                                      xt[t][:, h * 16:(h + 1) * 16])
            else:
                nc.sync.dma_start(x_sb[:], xt[t])
            return x_sb

        # All 8 x tiles are loaded upfront and stay resident (8MB SBUF):
        # the loads then finish in the first ~30us at full rate, leaving
        # the whole back half of the run's HBM bandwidth to the stores.
        xs = {t: load_tile(t) for t in range(n_btiles)}
        o_tiles = {}

        # Tiny Pool op that reads the LAST x tile: every SWDGE store is
        # queued behind it, so the store stream provably cannot start
        # until all loads have landed — stores never steal SDMA packet
        # slots from the compute-pacing loads.
        store_gate = const_pool.tile([1, 1], BF16)
        nc.gpsimd.tensor_copy(out=store_gate[:],
                              in_=xs[n_btiles - 2][0:1, N_PAIRS - 1, 127:128])

        for g in range(total):
            t, u = divmod(g, GROUPS)
            # ACT drains ~2/3 of groups (plain copies; DVE adds the bias
            # afterwards in cheap all-SBUF 2x mode); DVE drains the rest
            # with the bias fused.  Tile 0 is all-ACT so draining starts
            # before the bias constant has even landed.
            act_group = (t == 0) or (g % 4 in (1, 3)) or (g % 8 == 4)
            c0 = u * 1024
            if u == 0:
                o_tiles[t] = out_pool.tile([128, CO], BF16, tag="o_sb",
                                           name=f"o_sb_{t}")
            o_sb = o_tiles[t]
            ps_o = psum_o.tile([128, 1024], F32, tag="ps_o",
                               name=f"ps_o_{g}")
            for j in range(8):
                p = 8 * u + j  # pair index within tile
                nc.tensor.matmul(ps_o[:, j * 128:(j + 1) * 128],
                                 lhsT=xs[t][:, p],
                                 rhs=w_all[:, p],
                                 start=True, stop=True,
                                 skip_group_check=True)
            if act_group:
                # ACT drains the PSUM group; DVE then adds the bias
                # in-place in SBUF — all-bf16 all-SBUF, so it runs in
                # DVE 2x mode, about half the cost of a fused PSUM add.
                nc.scalar.copy(o_sb[:, c0:c0 + 1024], ps_o[:])
                nc.vector.tensor_add(out=o_sb[:, c0:c0 + 1024],
                                     in0=o_sb[:, c0:c0 + 1024],
                                     in1=bias_sb[:, c0:c0 + 1024])
            else:
                nc.vector.tensor_add(out=o_sb[:, c0:c0 + 1024],
                                     in0=ps_o[:],
                                     in1=bias_sb[:, c0:c0 + 1024])
            # Stores, whole tiles, split across BOTH rings: even tiles on
            # the SP HWDGE ring — whose FIFO queues them BEHIND all nine
            # upfront loads, so stores can never steal SDMA packet slots
            # from the compute-pacing loads — and odd tiles on SWDGE
            # (whose ~10us warmup overlaps the load phase).  Two parallel
            # store streams then drain the tail.  Quarters for the final
            # tile so it trickles out with the last drains.
            if t < n_btiles - 1:
                if u == GROUPS - 1:
                    # Tile 6 rides the SP ring (idle once loads finish),
                    # so the last tiles' bytes don't queue at the back
                    # of the SWDGE store FIFO.
                    eng = nc.gpsimd if t < 6 else nc.sync
                    eng.dma_start(out_t[t], o_sb[:])
            else:
                nc.sync.dma_start(out_t[t][:, c0:c0 + 1024],
                                  o_sb[:, c0:c0 + 1024])

    nc.compile()
    return nc


_NC_CACHE = {}


def _get_nc():
    if "nc" not in _NC_CACHE:
        _NC_CACHE["nc"] = _build_nc()
    return _NC_CACHE["nc"]


def _install_ntff_shim():
    """Profiling only: register the axon NTFF hook under antenv.axon_hooks.

    The container's antenv stub lacks axon_hooks, so bass_utils'
    `from antenv.axon_hooks import get_axon_ntff_profile_hook` raises on
    trace=True runs. Recreate the module from trn_agent_boot's ctypes hook.
    """
    import sys
    import types

    if "antenv.axon_hooks" in sys.modules:
        return
    from trn_agent_boot.trn_boot import _ntff_profile_via_ctypes

    hook = _ntff_profile_via_ctypes("/opt/axon/libaxon_pjrt.so")
    mod = types.ModuleType("antenv.axon_hooks")
    mod.get_axon_ntff_profile_hook = lambda: hook
    mod.set_axon_ntff_profile_hook = lambda h: None
    sys.modules["antenv.axon_hooks"] = mod
    import antenv

    antenv.axon_hooks = mod


def kernel(x, kernel, bias, _trace=False, _trace_kwargs=None):
    x = np.ascontiguousarray(x, dtype=np.float32)
    kernel = np.ascontiguousarray(kernel, dtype=np.float32)
    bias = np.ascontiguousarray(bias, dtype=np.float32)
    assert x.shape == (B, C, IN)

    if _trace:
        _install_ntff_shim()
    nc = _get_nc()

    # bf16 cast + per-b-tile transpose: xt[s, t, p, j, b] = shard s's
    # x[128t+b, 128j+p], so lhsT tiles stream straight from HBM.
    xb = x.reshape(N_CORES, N_BTILES, 128, N_PAIRS, 128).astype(NP_BF16)
    xtb = np.ascontiguousarray(xb.transpose(0, 1, 4, 3, 2))
    # Block-diagonal bf16 weight stacks: wall[p, j, :] holds cat 2j's
    # [i, o] block at [0:64, 0:64] and cat 2j+1's at [64:128, 64:128].
    wall = np.zeros((128, N_PAIRS, 128), dtype=np.float32)
    wall[0:IN, :, 0:OUT] = kernel[0, 0::2].transpose(1, 0, 2)
    wall[IN:128, :, OUT:128] = kernel[0, 1::2].transpose(1, 0, 2)
    wall = wall.astype(NP_BF16)
    biasf = np.ascontiguousarray(
        np.broadcast_to(bias.reshape(1, CO), (128, CO))).astype(NP_BF16)
    in_maps = [
        {
            "xt": xtb[i],
            "wall": wall,
            "biasf": biasf,
        }
        for i in range(N_CORES)
    ]
    res = run_bass_kernel_spmd(
        nc, in_maps, core_ids=list(range(N_CORES)),
        trace=_trace, **(_trace_kwargs or {})
    )
    out = np.concatenate(
        [np.asarray(res.results[i]["out"]) for i in range(N_CORES)], axis=0
    ).astype(np.float32)
    if _trace:
        _NC_CACHE["last_results"] = res
    return out


# revision 33
# speedup vs baseline: 1.1085x; 1.1085x over previous
"""CategoryDense (nn_CategoryDense) TRN2 Bass kernel — bf16 with
host-side transpose layout and phase-separated DMA.

out[b, c, o] = sum_i x[b, c, i] * kernel[0, c, i, o] + bias[0, c, o]
x: [8192, 64, 64] f32; kernel: [1, 64, 64, 64]; bias: [1, 64, 64].

Data-parallel over 8 NeuronCores: batch dim sharded 1024 rows/core,
weights + bias replicated; no cross-core communication.

The problem is HBM-bound. All device I/O is bf16 (host casts x down and
the result back up; rel-err budget 2e-2, ~5.6e-3 measured), halving DMA
bytes vs f32 — and bf16 matmuls run 1 cycle/row on PE where f32r needs
4 at 128-wide output.  The host upload stores x pre-transposed per
128-row b-tile as xt[t, p, j, b] = x[128t+b, 128j+p] (p = contraction
index of category pair j), so matmul lhsT tiles stream straight from
HBM at full descriptor size — no PE transposes, no PSUM round trip.

Device structure per core (8 b-tiles x 4 [128,1024] 2-bank PSUM
groups; all x and out tiles SBUF-resident):
  - PE runs ONLY the 256 [128,128] bf16 matmuls vs the block-diagonal
    weight stacks (the DMA feed paces PE at its mid p-state, so every
    extra PE cycle costs double — bias never touches PE).
  - Drains: ~2/3 of groups ACT-copy PSUM->SBUF with DVE adding bias
    afterwards in cheap all-SBUF 2x mode; the rest are fused DVE
    PSUM adds.  Bias arrives HOST-pre-broadcast as a plain [128, CO]
    load (an on-chip partition_broadcast is a 7.5us serial GPSIMD op
    that late-gates every drain).
  - DMA is strictly phase-separated: all 8 x tiles load upfront on the
    SP HWDGE ring at full bandwidth; stores ride SWDGE behind a tiny
    Pool gate that reads the second-to-last x tile, so the store
    stream provably cannot contend with the compute-pacing loads.
    Tiles 6-7 store via the (by then idle) SP ring so the last bytes
    don't queue at the back of the SWDGE FIFO.
"""

from contextlib import ExitStack

import numpy as np
import ml_dtypes

import concourse.bass as bass  # noqa: F401  (engine namespaces live on nc)
import concourse.mybir as mybir
import concourse.tile as tile
from concourse import bacc
from concourse.bass_utils import run_bass_kernel_spmd

F32 = mybir.dt.float32
BF16 = mybir.dt.bfloat16
NP_BF16 = ml_dtypes.bfloat16

N_CORES = 8
B, C, IN, OUT = 8192, 64, 64, 64
B_SHARD = B // N_CORES
N_PAIRS = C // 2          # 32 category pairs; one [128,128] matmul each
CI = C * IN               # 4096
CO = C * OUT              # 4096
N_BTILES = B_SHARD // 128  # 8
GROUPS = 4                # [128,1024] PSUM groups per b-tile (8 pairs each)
SKEW = 2                  # matmul emission lag behind psum alloc/prefill
# Units g with g % 8 in ACT_PHASE are ACT-drained (bias prefilled by
# ACT), the rest DVE-drained (bias added inline).  12/32 on ACT.
ACT_PHASE = (1, 4, 6)


def _build_nc(b_shard=B_SHARD):
    n_btiles = b_shard // 128
    total = n_btiles * GROUPS
    nc = bacc.Bacc("TRN2", target_bir_lowering=False, debug=False)
    # Host-pre-transposed x: xt[t, p, j, b] = x[128t+b, 128j+p].
    xt = nc.dram_tensor("xt", [n_btiles, 128, N_PAIRS, 128], BF16,
                        kind="ExternalInput").ap()
    # Host-prepared block-diagonal weight stacks (see kernel() below).
    wall = nc.dram_tensor("wall", [128, N_PAIRS, 128], BF16,
                          kind="ExternalInput").ap()
    # Host-pre-broadcast bias: [128, CO], identical rows.  A plain 1MB
    # HWDGE load; the on-chip partition_broadcast alternative is a
    # 7.5us serial GPSIMD op that late-gates every drain behind it.
    biasf = nc.dram_tensor("biasf", [128, CO], BF16,
                           kind="ExternalInput").ap()
    out = nc.dram_tensor("out", [b_shard, C, OUT], BF16,
                         kind="ExternalOutput").ap()

    out_t = out.rearrange("(t p) c o -> t p (c o)", p=128)

    with tile.TileContext(nc) as tc, ExitStack() as ctx:
        const_pool = ctx.enter_context(tc.tile_pool(name="const", bufs=1))
        x_pool = ctx.enter_context(tc.tile_pool(name="x", bufs=8))
        out_pool = ctx.enter_context(tc.tile_pool(name="out", bufs=8))
        # Four 2-bank PSUM tiles = all 8 banks.
        psum_o = ctx.enter_context(
            tc.tile_pool(name="psum_o", bufs=4, space="PSUM"))

        # Constants on the ACT HWDGE ring.  Few, large DMAs: each
        # dma_start costs ~650ns of sequencer + sem-lane time, and the 8
        # HWDGE completion lanes are shared with the load ring, so a
        # chain of small const DMAs head-of-line blocks the x loads.
        w_all = const_pool.tile([128, N_PAIRS, 128], BF16)
        for k in range(2):
            nc.scalar.dma_start(w_all[:, 16 * k:16 * (k + 1)],
                                wall[:, 16 * k:16 * (k + 1)])
        # Bias last: the first matmuls need w (and x0), while tile 0's
        # drains are bias-free ACT copies — bias isn't read until ~25us.
        bias_sb = const_pool.tile([128, CO], BF16)
        nc.scalar.dma_start(bias_sb[:], biasf[:])

        def load_tile(t):
            x_sb = x_pool.tile([128, N_PAIRS, 128], BF16, tag="x_sb",
                               name=f"x_sb_{t}")
            if t == 0:
                for h in range(2):
                    nc.sync.dma_start(x_sb[:, h * 16:(h + 1) * 16],
                                      xt[t][:, h * 16:(h + 1) * 16])
            else:
                nc.sync.dma_start(x_sb[:], xt[t])
            return x_sb

        # All 8 x tiles are loaded upfront and stay resident (8MB SBUF):
        # the loads then finish in the first ~30us at full rate, leaving
        # the whole back half of the run's HBM bandwidth to the stores.
        xs = {t: load_tile(t) for t in range(n_btiles)}
        o_tiles = {}

        # Tiny Pool op that reads the LAST x tile: every SWDGE store is
        # queued behind it, so the store stream provably cannot start
        # until all loads have landed — stores never steal SDMA packet
        # slots from the compute-pacing loads.
        store_gate = const_pool.tile([1, 1], BF16)
        nc.gpsimd.tensor_copy(out=store_gate[:],
                              in_=xs[n_btiles - 4][0:1, N_PAIRS - 1, 127:128])

        for g in range(total):
            t, u = divmod(g, GROUPS)
            # ACT drains ~2/3 of groups (plain copies; DVE adds the bias
            # afterwards in cheap all-SBUF 2x mode); DVE drains the rest
            # with the bias fused.  Tile 0 is all-ACT so draining starts
            # before the bias constant has even landed.
            act_group = (t == 0) or (g % 4 in (1, 3)) or (g % 8 == 4)
            c0 = u * 1024
            if u == 0:
                o_tiles[t] = out_pool.tile([128, CO], BF16, tag="o_sb",
                                           name=f"o_sb_{t}")
            o_sb = o_tiles[t]
            ps_o = psum_o.tile([128, 1024], F32, tag="ps_o",
                               name=f"ps_o_{g}")
            for j in range(8):
                p = 8 * u + j  # pair index within tile
                nc.tensor.matmul(ps_o[:, j * 128:(j + 1) * 128],
                                 lhsT=xs[t][:, p],
                                 rhs=w_all[:, p],
                                 start=True, stop=True,
                                 skip_group_check=True)
            if act_group:
                # ACT drains the PSUM group; DVE then adds the bias
                # in-place in SBUF — all-bf16 all-SBUF, so it runs in
                # DVE 2x mode, about half the cost of a fused PSUM add.
                nc.scalar.copy(o_sb[:, c0:c0 + 1024], ps_o[:])
                nc.vector.tensor_add(out=o_sb[:, c0:c0 + 1024],
                                     in0=o_sb[:, c0:c0 + 1024],
                                     in1=bias_sb[:, c0:c0 + 1024])
            else:
                nc.vector.tensor_add(out=o_sb[:, c0:c0 + 1024],
                                     in0=ps_o[:],
                                     in1=bias_sb[:, c0:c0 + 1024])
            # Stores, whole tiles, split across BOTH rings: even tiles on
            # the SP HWDGE ring — whose FIFO queues them BEHIND all nine
            # upfront loads, so stores can never steal SDMA packet slots
            # from the compute-pacing loads — and odd tiles on SWDGE
            # (whose ~10us warmup overlaps the load phase).  Two parallel
            # store streams then drain the tail.  Quarters for the final
            # tile so it trickles out with the last drains.
            if t < n_btiles - 1:
                if u == GROUPS - 1:
                    # Tile 6 rides the SP ring (idle once loads finish),
                    # so the last tiles' bytes don't queue at the back
                    # of the SWDGE store FIFO.
                    eng = nc.gpsimd if t < 5 else nc.sync
                    eng.dma_start(out_t[t], o_sb[:])
            else:
                nc.sync.dma_start(out_t[t][:, c0:c0 + 1024],
                                  o_sb[:, c0:c0 + 1024])

    nc.compile()
    return nc


_NC_CACHE = {}


def _get_nc():
    if "nc" not in _NC_CACHE:
        _NC_CACHE["nc"] = _build_nc()
    return _NC_CACHE["nc"]


def _install_ntff_shim():
    """Profiling only: register the axon NTFF hook under antenv.axon_hooks.

    The container's antenv stub lacks axon_hooks, so bass_utils'
    `from antenv.axon_hooks import get_axon_ntff_profile_hook` raises on
    trace=True runs. Recreate the module from trn_agent_boot's ctypes hook.
    """
    import sys
    import types

    if "antenv.axon_hooks" in sys.modules:
        return
    from trn_agent_boot.trn_boot import _ntff_profile_via_ctypes

    hook = _ntff_profile_via_ctypes("/opt/axon/libaxon_pjrt.so")
    mod = types.ModuleType("antenv.axon_hooks")
    mod.get_axon_ntff_profile_hook = lambda: hook
    mod.set_axon_ntff_profile_hook = lambda h: None
    sys.modules["antenv.axon_hooks"] = mod
    import antenv

    antenv.axon_hooks = mod


def kernel(x, kernel, bias, _trace=False, _trace_kwargs=None):
    x = np.ascontiguousarray(x, dtype=np.float32)
    kernel = np.ascontiguousarray(kernel, dtype=np.float32)
    bias = np.ascontiguousarray(bias, dtype=np.float32)
    assert x.shape == (B, C, IN)

    if _trace:
        _install_ntff_shim()
    nc = _get_nc()

    # bf16 cast + per-b-tile transpose: xt[s, t, p, j, b] = shard s's
    # x[128t+b, 128j+p], so lhsT tiles stream straight from HBM.
    xb = x.reshape(N_CORES, N_BTILES, 128, N_PAIRS, 128).astype(NP_BF16)
    xtb = np.ascontiguousarray(xb.transpose(0, 1, 4, 3, 2))
    # Block-diagonal bf16 weight stacks: wall[p, j, :] holds cat 2j's
    # [i, o] block at [0:64, 0:64] and cat 2j+1's at [64:128, 64:128].
    wall = np.zeros((128, N_PAIRS, 128), dtype=np.float32)
    wall[0:IN, :, 0:OUT] = kernel[0, 0::2].transpose(1, 0, 2)
    wall[IN:128, :, OUT:128] = kernel[0, 1::2].transpose(1, 0, 2)
    wall = wall.astype(NP_BF16)
    biasf = np.ascontiguousarray(
        np.broadcast_to(bias.reshape(1, CO), (128, CO))).astype(NP_BF16)
    in_maps = [
        {
            "xt": xtb[i],
            "wall": wall,
            "biasf": biasf,
        }
        for i in range(N_CORES)
    ]
    res = run_bass_kernel_spmd(
        nc, in_maps, core_ids=list(range(N_CORES)),
        trace=_trace, **(_trace_kwargs or {})
    )
    out = np.concatenate(
        [np.asarray(res.results[i]["out"]) for i in range(N_CORES)], axis=0
    ).astype(np.float32)
    if _trace:
        _NC_CACHE["last_results"] = res
    return out
